# revision 5
# baseline (speedup 1.0000x reference)
"""Trainium2 Bass kernel for nn_ConexaoRegional.

Reference computation:
    out[b, n, d, s] = sum_r xd[b, n, r] * wd[n, d, s, r]
where
    xd[b, (i,j), r] = x[b, 0, 4i+r, 4j+r]     (patch diagonal)
    wd[n, d, s, r]  = pesos[n, d, s, r, r]    (weight diagonal)

Shapes: x [64,1,128,128] f32, pesos [1024,16,32,4,4] f32,
        out [64,1024,16,32] f32 (128 MiB -> memory-bound).

Strategy: shard the region axis (n) across 8 cores (128 regions each).
Host packs, per core and per pair of regions (2p, 2p+1), a block-
diagonal stationary operand with rows (c, r) and cols (c*64+b), plus a
moving operand [8, 512] with the matching wd rows. Inputs are single
bf16 (product error ~4e-3, fp32 PSUM accumulation); output is stored
as fp16 (+5e-4) -- ~5e-3 total against the 2e-2 harness gate, halving
store traffic vs f32.

Pipeline: compute is fully decoupled from stores. The whole per-core
output (64 KiB/partition fp16) stages in SBUF; matmuls run back-to-back
(2 pairs per 2-bank PSUM tile), drained by Vector/Scalar copies that
cast f32->fp16. Store DMAs stream out with a tapered chunk schedule
(small first chunk to start the HBM stream early, large middle chunks
for DMA efficiency, small last chunk to shorten the tail). Dummy
warm-up matmuls run during the input-load ramp so the PE's HAM clock
gate reaches 2.4 GHz before real work arrives.
"""

import numpy as np

B = 64
R = 4
GH = GW = 32
N = GH * GW            # 1024 regions
D, S = 16, 32
DS = D * S             # 512
NCORES = 8
NPC = N // NCORES      # 128 regions per core
PAIRS = NPC // 2       # 64 pair-matmuls per core

# Store taper: pairs per store chunk (sums to PAIRS). Small head chunk
# starts the HBM store stream early; small tail chunk shortens the
# critical-path tail; big middle chunks amortize DMA overhead.
STORE_PAIRS = [2, 4, 6, 8, 10, 10, 10, 8, 4, 2]
assert sum(STORE_PAIRS) == PAIRS
N_WARMUP = 11          # dummy matmuls to warm the PE clock gate

_NC_CACHE = {}


def _build_bass():
    if "nc" in _NC_CACHE:
        return _NC_CACHE["nc"]
    from contextlib import ExitStack

    import concourse.bacc as bacc
    import concourse.mybir as mybir
    import concourse.tile as tile

    f32 = mybir.dt.float32
    f16 = mybir.dt.float16
    bf16 = mybir.dt.bfloat16
    nc = bacc.Bacc()  # Bacc (not raw Bass): its compile passes split multi-sem
    # waits and move matmul waits to ldweights, which TRN2 codegen requires.

    # K = 8 rows: (c, r) with c in {0,1} regions per pair, r in 0..3.
    xbd = nc.declare_dram_parameter("xbd", [8, PAIRS * 128], bf16, isOutput=False)
    wt = nc.declare_dram_parameter("wt", [8, PAIRS * DS], bf16, isOutput=False)
    out = nc.declare_dram_parameter("out", [PAIRS * 128 * DS], f16, isOutput=True)

    with ExitStack() as ctx:
        tc = ctx.enter_context(tile.TileContext(nc))
        const = ctx.enter_context(tc.tile_pool(name="const", bufs=1))
        wupool = ctx.enter_context(tc.tile_pool(name="wu", bufs=1, space="PSUM"))
        pspool = ctx.enter_context(tc.tile_pool(name="ps", bufs=3, space="PSUM"))

        # All loads issue up front; first slices are small so pair 0 can
        # start as soon as possible after the DGE pipeline latency.
        wsb = const.tile([8, PAIRS * DS], bf16)
        xsb = const.tile([8, PAIRS * 128], bf16)
        whead = 4 * DS
        xhead = 4 * 128
        nc.sync.dma_start(wsb[:, :whead], wt[:, :whead])
        nc.sync.dma_start(xsb[:, :xhead], xbd[:, :xhead])
        nc.sync.dma_start(wsb[:, whead:], wt[:, whead:])
        nc.sync.dma_start(xsb[:, xhead:], xbd[:, xhead:])

        # Whole-output staging buffer: 64 KiB/partition fp16.
        osb = const.tile([128, PAIRS * DS], f16)

        # Warm-up: back-to-back dummy matmuls on garbage SBUF data keep the
        # PE busy through the load ramp so HAM un-throttles to 2.4 GHz.
        for w in range(N_WARMUP):
            wps = wupool.tile([128, DS], f32, name="wu", tag="wu")
            nc.tensor.matmul(
                wps[:],
                lhsT=xsb[:, 0:128],
                rhs=wsb[:, 0:DS],
                start=True,
                stop=True,
            )

        # Main stream: 2 pairs per 2-bank PSUM tile, one draining copy each,
        # alternating Vector/Scalar. Store chunk s issues right after its
        # last copy; the Tile scheduler overlaps everything by dependency.
        base_pair = 0
        base_elem = 0
        for s, npair in enumerate(STORE_PAIRS):
            for jj in range(npair // 2):
                ps = pspool.tile([128, 2 * DS], f32)
                for c2 in range(2):
                    p = base_pair + jj * 2 + c2
                    nc.tensor.matmul(
                        ps[:, c2 * DS:(c2 + 1) * DS],
                        lhsT=xsb[:, p * 128:(p + 1) * 128],
                        rhs=wsb[:, p * DS:(p + 1) * DS],
                        start=True,
                        stop=True,
                    )
                p0 = base_pair + jj * 2
                dst = osb[:, p0 * DS:(p0 + 2) * DS]
                if (p0 // 2) % 2 == 0:
                    nc.vector.tensor_copy(dst, ps[:])
                else:
                    nc.scalar.copy(dst, ps[:])
            nelem = npair * 128 * DS
            nc.sync.dma_start(
                out[base_elem:base_elem + nelem],
                osb[:, base_pair * DS:(base_pair + npair) * DS],
            )
            base_pair += npair
            base_elem += nelem

    nc.compile()  # Bacc passes: reg alloc, wait splitting, ldweights fixup
    _NC_CACHE["nc"] = nc
    return nc


def _pack_inputs(x, pesos):
    import ml_dtypes

    bf16 = ml_dtypes.bfloat16
    x = np.ascontiguousarray(np.asarray(x), dtype=np.float32)
    pesos = np.ascontiguousarray(np.asarray(pesos), dtype=np.float32)
    # xd[b, i, j, r] = x[b, 0, 4i+r, 4j+r]
    xp = x.reshape(B, GH, R, GW, R)
    xd = np.einsum("birjr->bijr", xp).reshape(B, N, R)
    # wd[n, ds, r] = pesos[n, d, s, r, r]
    wd = pesos.reshape(N, DS, R * R)[:, :, :: R + 1]  # [N, 512, 4]

    in_maps = []
    for k in range(NCORES):
        n0 = k * NPC
        xdk = xd[:, n0:n0 + NPC, :]   # [B, 128, 4]
        wdk = wd[n0:n0 + NPC]         # [128, 512, 4]
        # wt[c*4+r, p*512+ds] = wdk[2p+c, ds, r]
        wtk = np.ascontiguousarray(
            wdk.reshape(PAIRS, 2, DS, R).transpose(1, 3, 0, 2)
        ).reshape(8, PAIRS * DS).astype(bf16)
        # xbd[c*4+r, p*128 + c*64 + b] = xdk[b, 2p+c, r] (block-diagonal)
        A = xdk.reshape(B, PAIRS, 2, R).transpose(2, 3, 1, 0)  # [c, r, p, b]
        L = np.zeros((2, R, PAIRS, 2, B), dtype=np.float32)
        L[0, :, :, 0, :] = A[0]
        L[1, :, :, 1, :] = A[1]
        xbdk = L.reshape(8, PAIRS * 128).astype(bf16)
        in_maps.append(
            {
                "xbd": np.ascontiguousarray(xbdk),
                "wt": np.ascontiguousarray(wtk),
            }
        )
    return in_maps


TRACE = {"on": False, "last": None}


def kernel(x, pesos):
    from concourse.bass_utils import run_bass_kernel_spmd

    in_maps = _pack_inputs(x, pesos)
    nc = _build_bass()
    res = run_bass_kernel_spmd(
        nc, in_maps, core_ids=list(range(NCORES)), trace=TRACE["on"]
    )
    TRACE["last"] = res
    outs = []
    for k in range(NCORES):
        # Each store chunk s is [128=(c*64+b), npair*512] row-major in DRAM.
        flat = res.results[k]["out"]
        parts = []
        base_pair = 0
        base_elem = 0
        for npair in STORE_PAIRS:
            nelem = npair * 128 * DS
            arr = flat[base_elem:base_elem + nelem].astype(np.float32)
            arr = arr.reshape(2, B, npair, DS).transpose(1, 2, 0, 3)
            parts.append(arr)  # [B, npair, 2, DS]
            base_pair += npair
            base_elem += nelem
        core = np.concatenate(parts, axis=1).reshape(B, NPC, DS)
        outs.append(core)
    full = np.concatenate(outs, axis=1)  # [B, N, DS]
    return np.ascontiguousarray(full).reshape(B, N, D, S)


# revision 7
# speedup vs baseline: 1.0563x; 1.0563x over previous
"""Trainium2 Bass kernel for nn_ConexaoRegional.

Reference computation:
    out[b, n, d, s] = sum_r xd[b, n, r] * wd[n, d, s, r]
where
    xd[b, (i,j), r] = x[b, 0, 4i+r, 4j+r]     (patch diagonal)
    wd[n, d, s, r]  = pesos[n, d, s, r, r]    (weight diagonal)

Shapes: x [64,1,128,128] f32, pesos [1024,16,32,4,4] f32,
        out [64,1024,16,32] f32 (128 MiB -> memory-bound).

Strategy: shard the region axis (n) across 8 cores (128 regions each).
Host packs, per core and per pair of regions (2p, 2p+1), a block-
diagonal stationary operand with rows (c, r) and cols (c*64+b), plus a
moving operand [8, 512] with the matching wd rows. Inputs are single
bf16 (product error ~4e-3, fp32 PSUM accumulation); output is stored
as fp16 (+5e-4) -- ~5e-3 total against the 2e-2 harness gate, halving
store traffic vs f32.

Pipeline: compute is fully decoupled from stores. The whole per-core
output (64 KiB/partition fp16) stages in SBUF; matmuls run back-to-back
(2 pairs per 2-bank PSUM tile), drained by Vector/Scalar copies that
cast f32->fp16. Store DMAs stream out with a tapered chunk schedule
(small first chunk to start the HBM stream early, large middle chunks
for DMA efficiency, small last chunk to shorten the tail). Dummy
warm-up matmuls run during the input-load ramp so the PE's HAM clock
gate reaches 2.4 GHz before real work arrives.
"""

import numpy as np

B = 64
R = 4
GH = GW = 32
N = GH * GW            # 1024 regions
D, S = 16, 32
DS = D * S             # 512
NCORES = 8
NPC = N // NCORES      # 128 regions per core
PAIRS = NPC // 2       # 64 pair-matmuls per core

# Store taper: pairs per store chunk (sums to PAIRS). Small head chunk
# starts the HBM store stream early; small tail chunk shortens the
# critical-path tail; big middle chunks amortize DMA overhead.
STORE_PAIRS = [2, 4, 6, 8, 10, 10, 10, 8, 4, 2]
assert sum(STORE_PAIRS) == PAIRS
N_WARMUP = 8           # dummy matmuls to warm the PE clock gate

_NC_CACHE = {}


def _build_bass():
    if "nc" in _NC_CACHE:
        return _NC_CACHE["nc"]
    from contextlib import ExitStack

    import concourse.bacc as bacc
    import concourse.mybir as mybir
    import concourse.tile as tile

    f32 = mybir.dt.float32
    f16 = mybir.dt.float16
    bf16 = mybir.dt.bfloat16
    nc = bacc.Bacc()  # Bacc (not raw Bass): its compile passes split multi-sem
    # waits and move matmul waits to ldweights, which TRN2 codegen requires.

    # K = 8 rows: (c, r) with c in {0,1} regions per pair, r in 0..3.
    xbd = nc.declare_dram_parameter("xbd", [8, PAIRS * 128], bf16, isOutput=False)
    wt = nc.declare_dram_parameter("wt", [8, PAIRS * DS], bf16, isOutput=False)
    out = nc.declare_dram_parameter("out", [PAIRS * 128 * DS], f16, isOutput=True)

    with ExitStack() as ctx:
        tc = ctx.enter_context(tile.TileContext(nc))
        const = ctx.enter_context(tc.tile_pool(name="const", bufs=1))
        wupool = ctx.enter_context(tc.tile_pool(name="wu", bufs=1, space="PSUM"))
        pspool = ctx.enter_context(tc.tile_pool(name="ps", bufs=3, space="PSUM"))

        # All loads issue up front, split into many slices so completion
        # semaphores fire incrementally (the compute stream consumes pairs
        # in order and must never wait on a monolithic load). [8, N] tiles
        # reach only ~2 DMA engines (~55 GB/s), so supply rate ~7 pairs/us
        # still outpaces consumption (~2.4 pairs/us). Issues spread across
        # sync/scalar/gpsimd sequencers to keep the sync queue free for
        # store issues.
        wsb = const.tile([8, PAIRS * DS], bf16)
        xsb = const.tile([8, PAIRS * 128], bf16)

        def wslice(a, b):
            return wsb[:, a * DS:b * DS], wt[:, a * DS:b * DS]

        def xslice(a, b):
            return xsb[:, a * 128:b * 128], xbd[:, a * 128:b * 128]

        nc.sync.dma_start(*xslice(0, 4))
        nc.sync.dma_start(*wslice(0, 4))
        nc.sync.dma_start(*wslice(4, 12))
        nc.scalar.dma_start(*wslice(12, 20))
        nc.scalar.dma_start(*wslice(20, 28))
        nc.gpsimd.dma_start(*xslice(4, 32))
        nc.gpsimd.dma_start(*xslice(32, 64))
        nc.gpsimd.dma_start(*wslice(28, 40))
        nc.gpsimd.dma_start(*wslice(40, 52))
        nc.gpsimd.dma_start(*wslice(52, 64))

        # Whole-output staging buffer: 64 KiB/partition fp16.
        osb = const.tile([128, PAIRS * DS], f16)

        # Warm-up: back-to-back dummy matmuls on garbage SBUF data keep the
        # PE busy through the load ramp so HAM un-throttles to 2.4 GHz.
        for w in range(N_WARMUP):
            wps = wupool.tile([128, DS], f32, name="wu", tag="wu")
            nc.tensor.matmul(
                wps[:],
                lhsT=xsb[:, 0:128],
                rhs=wsb[:, 0:DS],
                start=True,
                stop=True,
            )

        # Main stream: 2 pairs per 2-bank PSUM tile, one draining copy each,
        # alternating Vector/Scalar. Store chunk s issues right after its
        # last copy; the Tile scheduler overlaps everything by dependency.
        base_pair = 0
        base_elem = 0
        for s, npair in enumerate(STORE_PAIRS):
            for jj in range(npair // 2):
                ps = pspool.tile([128, 2 * DS], f32)
                for c2 in range(2):
                    p = base_pair + jj * 2 + c2
                    nc.tensor.matmul(
                        ps[:, c2 * DS:(c2 + 1) * DS],
                        lhsT=xsb[:, p * 128:(p + 1) * 128],
                        rhs=wsb[:, p * DS:(p + 1) * DS],
                        start=True,
                        stop=True,
                    )
                p0 = base_pair + jj * 2
                dst = osb[:, p0 * DS:(p0 + 2) * DS]
                if (p0 // 2) % 2 == 0:
                    nc.vector.tensor_copy(dst, ps[:])
                else:
                    nc.scalar.copy(dst, ps[:])
            nelem = npair * 128 * DS
            nc.sync.dma_start(
                out[base_elem:base_elem + nelem],
                osb[:, base_pair * DS:(base_pair + npair) * DS],
            )
            base_pair += npair
            base_elem += nelem

    nc.compile()  # Bacc passes: reg alloc, wait splitting, ldweights fixup
    _NC_CACHE["nc"] = nc
    return nc


def _pack_inputs(x, pesos):
    import ml_dtypes

    bf16 = ml_dtypes.bfloat16
    x = np.ascontiguousarray(np.asarray(x), dtype=np.float32)
    pesos = np.ascontiguousarray(np.asarray(pesos), dtype=np.float32)
    # xd[b, i, j, r] = x[b, 0, 4i+r, 4j+r]
    xp = x.reshape(B, GH, R, GW, R)
    xd = np.einsum("birjr->bijr", xp).reshape(B, N, R)
    # wd[n, ds, r] = pesos[n, d, s, r, r]
    wd = pesos.reshape(N, DS, R * R)[:, :, :: R + 1]  # [N, 512, 4]

    in_maps = []
    for k in range(NCORES):
        n0 = k * NPC
        xdk = xd[:, n0:n0 + NPC, :]   # [B, 128, 4]
        wdk = wd[n0:n0 + NPC]         # [128, 512, 4]
        # wt[c*4+r, p*512+ds] = wdk[2p+c, ds, r]
        wtk = np.ascontiguousarray(
            wdk.reshape(PAIRS, 2, DS, R).transpose(1, 3, 0, 2)
        ).reshape(8, PAIRS * DS).astype(bf16)
        # xbd[c*4+r, p*128 + c*64 + b] = xdk[b, 2p+c, r] (block-diagonal)
        A = xdk.reshape(B, PAIRS, 2, R).transpose(2, 3, 1, 0)  # [c, r, p, b]
        L = np.zeros((2, R, PAIRS, 2, B), dtype=np.float32)
        L[0, :, :, 0, :] = A[0]
        L[1, :, :, 1, :] = A[1]
        xbdk = L.reshape(8, PAIRS * 128).astype(bf16)
        in_maps.append(
            {
                "xbd": np.ascontiguousarray(xbdk),
                "wt": np.ascontiguousarray(wtk),
            }
        )
    return in_maps


TRACE = {"on": False, "last": None}


def kernel(x, pesos):
    from concourse.bass_utils import run_bass_kernel_spmd

    in_maps = _pack_inputs(x, pesos)
    nc = _build_bass()
    res = run_bass_kernel_spmd(
        nc, in_maps, core_ids=list(range(NCORES)), trace=TRACE["on"]
    )
    TRACE["last"] = res
    outs = []
    for k in range(NCORES):
        # Each store chunk s is [128=(c*64+b), npair*512] row-major in DRAM.
        flat = res.results[k]["out"]
        parts = []
        base_pair = 0
        base_elem = 0
        for npair in STORE_PAIRS:
            nelem = npair * 128 * DS
            arr = flat[base_elem:base_elem + nelem].astype(np.float32)
            arr = arr.reshape(2, B, npair, DS).transpose(1, 2, 0, 3)
            parts.append(arr)  # [B, npair, 2, DS]
            base_pair += npair
            base_elem += nelem
        core = np.concatenate(parts, axis=1).reshape(B, NPC, DS)
        outs.append(core)
    full = np.concatenate(outs, axis=1)  # [B, N, DS]
    return np.ascontiguousarray(full).reshape(B, N, D, S)


# revision 8
# speedup vs baseline: 1.0860x; 1.0282x over previous
"""Trainium2 Bass kernel for nn_ConexaoRegional.

Reference computation:
    out[b, n, d, s] = sum_r xd[b, n, r] * wd[n, d, s, r]
where
    xd[b, (i,j), r] = x[b, 0, 4i+r, 4j+r]     (patch diagonal)
    wd[n, d, s, r]  = pesos[n, d, s, r, r]    (weight diagonal)

Shapes: x [64,1,128,128] f32, pesos [1024,16,32,4,4] f32,
        out [64,1024,16,32] f32 (128 MiB -> memory-bound).

Strategy: shard the region axis (n) across 8 cores (128 regions each,
64 region-pairs). Per pair j, a block-diagonal stationary operand
lhsT[8=(c,r), 128=(c*64+b)] against a moving operand rhs[8, 512=(d,s)]
produces the pair's full output tile [128, 512] in one matmul. Inputs
are single bf16 (product error ~4e-3, fp32 PSUM accumulation); output
is stored as fp16 (+5e-4) -- ~5e-3 total against the 2e-2 harness
gate, halving store traffic vs f32.

K=8 leaves the 128x128 PE array mostly idle and the HAM clock gate
keeps the PE at 1.2 GHz (the copy-gated duty cycle never fills a full
activity window), so matmuls are packed 4-concurrent with 4x row
tiling: pair j runs in array row-strip q = j%4 via tile_position=
(32q, 0), with its operands resident on SBUF partitions 32q..32q+7.
The x and w operands for each pair are interleaved in one DRAM row
block per strip so one contiguous DMA per (strip, block-range) feeds
both. Concurrent tiles write distinct PSUM banks (2-pair 2-bank
tiles); Vector/Scalar copies alternate draining them into a whole-
output SBUF staging buffer (64 KiB/partition fp16), casting f32->fp16.
Store DMAs stream out on a tapered chunk schedule (small first chunk
to start the HBM stream early, big middle chunks for DMA efficiency,
small last chunk to shorten the tail).
"""

import numpy as np

B = 64
R = 4
GH = GW = 32
N = GH * GW            # 1024 regions
D, S = 16, 32
DS = D * S             # 512
NCORES = 8
NPC = N // NCORES      # 128 regions per core
PAIRS = NPC // 2       # 64 pair-matmuls per core
NBLK = PAIRS // 4      # 16 blocks per row-strip
BLKW = 128 + DS        # interleaved x (128) + w (512) columns per pair

# Store taper: pairs per store chunk (sums to PAIRS).
STORE_PAIRS = [2, 4, 6, 8, 10, 10, 10, 8, 4, 2]
assert sum(STORE_PAIRS) == PAIRS

_NC_CACHE = {}


def _build_bass():
    if "nc" in _NC_CACHE:
        return _NC_CACHE["nc"]
    from contextlib import ExitStack

    import concourse.bacc as bacc
    import concourse.mybir as mybir
    import concourse.tile as tile

    f32 = mybir.dt.float32
    f16 = mybir.dt.float16
    bf16 = mybir.dt.bfloat16
    nc = bacc.Bacc()  # Bacc (not raw Bass): its compile passes split multi-sem
    # waits and move matmul waits to ldweights, which TRN2 codegen requires.

    # xw[8q+k, t*640+0:128]   = x block-diag for pair j=4t+q, K-row k=(c,r)
    # xw[8q+k, t*640+128:640] = w for pair j=4t+q, K-row k
    xw = nc.declare_dram_parameter("xw", [32, NBLK * BLKW], bf16, isOutput=False)
    out = nc.declare_dram_parameter("out", [PAIRS * 128 * DS], f16, isOutput=True)

    with ExitStack() as ctx:
        tc = ctx.enter_context(tile.TileContext(nc))
        const = ctx.enter_context(tc.tile_pool(name="const", bufs=1))
        pspool = ctx.enter_context(tc.tile_pool(name="ps", bufs=4, space="PSUM"))

        xwsb = const.tile([128, NBLK * BLKW], bf16)  # 20 KiB/partition
        osb = const.tile([128, PAIRS * DS], f16)     # 64 KiB/partition

        # Loads: per strip q, a small head (blocks 0-1) for a fast start,
        # then mid (2-7) and tail (8-15) slices so completion semaphores
        # fire incrementally. Issues spread across sync/scalar/gpsimd
        # sequencers; supply comfortably outpaces the copy-gated consume
        # rate (~3.4 pairs/us).
        def ld(issuer, q, t0, t1):
            issuer.dma_start(
                xwsb[32 * q:32 * q + 8, t0 * BLKW:t1 * BLKW],
                xw[8 * q:8 * q + 8, t0 * BLKW:t1 * BLKW],
            )

        for q in range(4):
            ld(nc.sync, q, 0, 2)
        for q in range(4):
            ld(nc.scalar, q, 2, 8)
        for q in range(4):
            ld(nc.gpsimd, q, 8, NBLK)

        # Main stream: 2 pairs per 2-bank PSUM tile; consecutive pairs sit
        # in different row strips, so 4 matmuls run concurrently in the
        # array (pairs j, j+1, j+2, j+3 -> strips 0..3, 4 distinct banks).
        base_pair = 0
        base_elem = 0
        for s, npair in enumerate(STORE_PAIRS):
            for jj in range(npair // 2):
                ps = pspool.tile([128, 2 * DS], f32)
                for c2 in range(2):
                    j = base_pair + jj * 2 + c2
                    q, t = j % 4, j // 4
                    col = t * BLKW
                    nc.tensor.matmul(
                        ps[:, c2 * DS:(c2 + 1) * DS],
                        lhsT=xwsb[32 * q:32 * q + 8, col:col + 128],
                        rhs=xwsb[32 * q:32 * q + 8, col + 128:col + BLKW],
                        tile_position=(32 * q, 0),
                        start=True,
                        stop=True,
                    )
                p0 = base_pair + jj * 2
                dst = osb[:, p0 * DS:(p0 + 2) * DS]
                if (p0 // 2) % 2 == 0:
                    nc.vector.tensor_copy(dst, ps[:])
                else:
                    nc.scalar.copy(dst, ps[:])
            nelem = npair * 128 * DS
            nc.sync.dma_start(
                out[base_elem:base_elem + nelem],
                osb[:, base_pair * DS:(base_pair + npair) * DS],
            )
            base_pair += npair
            base_elem += nelem

    nc.compile()  # Bacc passes: reg alloc, wait splitting, ldweights fixup
    _NC_CACHE["nc"] = nc
    return nc


def _pack_inputs(x, pesos):
    import ml_dtypes

    bf16 = ml_dtypes.bfloat16
    x = np.ascontiguousarray(np.asarray(x), dtype=np.float32)
    pesos = np.ascontiguousarray(np.asarray(pesos), dtype=np.float32)
    # xd[b, i, j, r] = x[b, 0, 4i+r, 4j+r]
    xp = x.reshape(B, GH, R, GW, R)
    xd = np.einsum("birjr->bijr", xp).reshape(B, N, R)
    # wd[n, ds, r] = pesos[n, d, s, r, r]
    wd = pesos.reshape(N, DS, R * R)[:, :, :: R + 1]  # [N, 512, 4]

    in_maps = []
    for k in range(NCORES):
        n0 = k * NPC
        xdk = xd[:, n0:n0 + NPC, :]   # [B, 128, 4]
        wdk = wd[n0:n0 + NPC]         # [128, 512, 4]
        # wtk[(c,r), j, ds] = wdk[2j+c, ds, r]
        wtk = wdk.reshape(PAIRS, 2, DS, R).transpose(1, 3, 0, 2)  # [c,r,j,ds]
        wtk = wtk.reshape(8, PAIRS, DS)
        # xbdk[(c,r), j, c'*64+b] = xdk[b, 2j+c, r] iff c==c'
        A = xdk.reshape(B, PAIRS, 2, R).transpose(2, 3, 1, 0)  # [c, r, j, b]
        L = np.zeros((2, R, PAIRS, 2, B), dtype=np.float32)
        L[0, :, :, 0, :] = A[0]
        L[1, :, :, 1, :] = A[1]
        xbdk = L.reshape(8, PAIRS, 128)
        # strip/block interleave: xw[q, kr, t, 0:128]=x, [128:640]=w for
        # pair j = 4t + q.
        xs = xbdk.reshape(8, NBLK, 4, 128).transpose(2, 0, 1, 3)  # [q,k,t,128]
        ws = wtk.reshape(8, NBLK, 4, DS).transpose(2, 0, 1, 3)    # [q,k,t,512]
        xwk = np.concatenate(
            [xs, ws.astype(np.float32)], axis=3
        ).reshape(32, NBLK * BLKW).astype(bf16)
        in_maps.append({"xw": np.ascontiguousarray(xwk)})
    return in_maps


TRACE = {"on": False, "last": None}


def kernel(x, pesos):
    from concourse.bass_utils import run_bass_kernel_spmd

    in_maps = _pack_inputs(x, pesos)
    nc = _build_bass()
    res = run_bass_kernel_spmd(
        nc, in_maps, core_ids=list(range(NCORES)), trace=TRACE["on"]
    )
    TRACE["last"] = res
    outs = []
    for k in range(NCORES):
        # Each store chunk s is [128=(c*64+b), npair*512] row-major in DRAM.
        flat = res.results[k]["out"]
        parts = []
        base_elem = 0
        for npair in STORE_PAIRS:
            nelem = npair * 128 * DS
            arr = flat[base_elem:base_elem + nelem].astype(np.float32)
            arr = arr.reshape(2, B, npair, DS).transpose(1, 2, 0, 3)
            parts.append(arr)  # [B, npair, 2, DS]
            base_elem += nelem
        core = np.concatenate(parts, axis=1).reshape(B, NPC, DS)
        outs.append(core)
    full = np.concatenate(outs, axis=1)  # [B, N, DS]
    return np.ascontiguousarray(full).reshape(B, N, D, S)


# revision 14
# speedup vs baseline: 1.2842x; 1.1825x over previous
"""Trainium2 Bass kernel for nn_ConexaoRegional.

Reference computation:
    out[b, n, d, s] = sum_r xd[b, n, r] * wd[n, d, s, r]
where
    xd[b, (i,j), r] = x[b, 0, 4i+r, 4j+r]     (patch diagonal)
    wd[n, d, s, r]  = pesos[n, d, s, r, r]    (weight diagonal)

Shapes: x [64,1,128,128] f32, pesos [1024,16,32,4,4] f32,
        out [64,1024,16,32] f32 (128 MiB -> memory-bound).

Strategy: shard the region axis (n) across 8 cores (128 regions each,
64 region-pairs). Per pair j, a block-diagonal stationary operand
lhsT[8=(c,r), 128=(c*64+b)] against a moving operand rhs[8, 512=(d,s)]
produces the pair's full output tile [128, 512] in one matmul. Inputs
are single bf16 (product error ~4e-3, fp32 PSUM accumulation); output
is stored as fp16 (+5e-4) -- ~5e-3 total against the 2e-2 harness
gate, halving store traffic vs f32.

K=8 leaves the 128x128 PE array mostly idle and the HAM clock gate
keeps the PE at 1.2 GHz (the copy-gated duty cycle never fills a full
activity window), so matmuls are packed 4-concurrent with 4x row
tiling: pair j runs in array row-strip q = j%4 via tile_position=
(32q, 0). Operands for pair j = q + 4u + 16t live on SBUF partitions
32q+8u .. 32q+8u+8 (4 substreams per strip), with x and w interleaved
per 640-column block t, so the whole input is a dense [128, 2560]
image loaded by just TWO full-width DMAs (a 160 KiB head covering the
first 16 pairs, then the rest) -- all 16 DMA engines, early completion
semaphores, no narrow-partition crawl.

Concurrent tiles write distinct PSUM banks (2-pair 2-bank tiles);
Vector/Scalar copies alternate draining them into a whole-output SBUF
staging buffer (64 KiB/partition fp16), casting f32->fp16. Store DMAs
stream out on a tapered chunk schedule (small first chunk to start the
HBM stream early, big middle chunks for DMA efficiency, small last
chunk to shorten the tail).
"""

import numpy as np

B = 64
R = 4
GH = GW = 32
N = GH * GW            # 1024 regions
D, S = 16, 32
DS = D * S             # 512
NCORES = 8
NPC = N // NCORES      # 128 regions per core
PAIRS = NPC // 2       # 64 pair-matmuls per core
BLKW = 128 + DS        # interleaved x (128) + w (512) columns per pair
NBLK = 16              # blocks t per strip; pair j = (j%4) + 4*(j//4)

# Store taper: pairs per store chunk (sums to PAIRS).
STORE_PAIRS = [2, 4, 6, 8, 10, 10, 10, 8, 4, 2]
assert sum(STORE_PAIRS) == PAIRS

_NC_CACHE = {}


def _build_bass():
    if "nc" in _NC_CACHE:
        return _NC_CACHE["nc"]
    from contextlib import ExitStack

    import concourse.bacc as bacc
    import concourse.mybir as mybir
    import concourse.tile as tile

    f32 = mybir.dt.float32
    f16 = mybir.dt.float16
    bf16 = mybir.dt.bfloat16
    nc = bacc.Bacc()  # Bacc (not raw Bass): its compile passes split multi-sem
    # waits and move matmul waits to ldweights, which TRN2 codegen requires.

    # xw[32q+k, t*640+0:128]   = x block-diag for pair j = 4t+q, row k=(c,r)
    # xw[32q+k, t*640+128:640] = w for pair j, row k. Rows 32q+8..32q+31 are
    # zero padding: matmul operands must sit at 32-aligned base partitions,
    # and a full-width [128, W] image loads on all 16 DMA engines (a dense
    # [8, W] layout would crawl at ~2 engines).
    xw = nc.declare_dram_parameter("xw", [128, NBLK * BLKW], bf16, isOutput=False)
    out = nc.declare_dram_parameter("out", [PAIRS * 128 * DS], f16, isOutput=True)

    with ExitStack() as ctx:
        tc = ctx.enter_context(tile.TileContext(nc))
        const = ctx.enter_context(tc.tile_pool(name="const", bufs=1))
        pspool = ctx.enter_context(tc.tile_pool(name="ps", bufs=4, space="PSUM"))

        xwsb = const.tile([128, NBLK * BLKW], bf16)  # 20 KiB/partition
        osb = const.tile([128, PAIRS * DS], f16)     # 64 KiB/partition

        # Full-width loads in 4 column slices (blocks [0:2, 2:6, 6:11,
        # 11:16]) so completion semaphores stay ahead of the copy-paced
        # consumption (~3.4 pairs/us = block every ~1.2us).
        for t0, t1 in ((0, 2), (2, 6), (6, 11), (11, NBLK)):
            nc.sync.dma_start(
                xwsb[:, t0 * BLKW:t1 * BLKW], xw[:, t0 * BLKW:t1 * BLKW]
            )

        # Main stream: 2 pairs per 2-bank PSUM tile; consecutive pairs sit
        # in different row strips, so 4 matmuls run concurrently in the
        # array (4 distinct PSUM banks in flight).
        base_pair = 0
        base_elem = 0
        for s, npair in enumerate(STORE_PAIRS):
            for jj in range(npair // 2):
                ps = pspool.tile([128, 2 * DS], f32)
                for c2 in range(2):
                    j = base_pair + jj * 2 + c2
                    q, t = j % 4, j // 4
                    row = 32 * q
                    col = t * BLKW
                    nc.tensor.matmul(
                        ps[:, c2 * DS:(c2 + 1) * DS],
                        lhsT=xwsb[row:row + 8, col:col + 128],
                        rhs=xwsb[row:row + 8, col + 128:col + BLKW],
                        tile_position=(32 * q, 0),
                        start=True,
                        stop=True,
                    )
                p0 = base_pair + jj * 2
                dst = osb[:, p0 * DS:(p0 + 2) * DS]
                if (p0 // 2) % 2 == 0:
                    nc.vector.tensor_copy(dst, ps[:])
                else:
                    nc.scalar.copy(dst, ps[:])
            nelem = npair * 128 * DS
            nc.sync.dma_start(
                out[base_elem:base_elem + nelem],
                osb[:, base_pair * DS:(base_pair + npair) * DS],
            )
            base_pair += npair
            base_elem += nelem

    nc.compile()  # Bacc passes: reg alloc, wait splitting, ldweights fixup
    _NC_CACHE["nc"] = nc
    return nc


def _pack_inputs(x, pesos):
    import ml_dtypes

    bf16 = ml_dtypes.bfloat16
    x = np.ascontiguousarray(np.asarray(x), dtype=np.float32)
    pesos = np.ascontiguousarray(np.asarray(pesos), dtype=np.float32)
    # xd[b, i, j, r] = x[b, 0, 4i+r, 4j+r]
    xp = x.reshape(B, GH, R, GW, R)
    xd = np.einsum("birjr->bijr", xp).reshape(B, N, R)
    # wd[n, ds, r] = pesos[n, d, s, r, r]
    wd = pesos.reshape(N, DS, R * R)[:, :, :: R + 1]  # [N, 512, 4]

    in_maps = []
    for k in range(NCORES):
        n0 = k * NPC
        xdk = xd[:, n0:n0 + NPC, :]   # [B, 128, 4]
        wdk = wd[n0:n0 + NPC]         # [128, 512, 4]
        # wtk[(c,r), j, ds] = wdk[2j+c, ds, r]
        wtk = wdk.reshape(PAIRS, 2, DS, R).transpose(1, 3, 0, 2)  # [c,r,j,ds]
        wtk = wtk.reshape(8, PAIRS, DS)
        # xbdk[(c,r), j, c'*64+b] = xdk[b, 2j+c, r] iff c==c'
        A = xdk.reshape(B, PAIRS, 2, R).transpose(2, 3, 1, 0)  # [c, r, j, b]
        L = np.zeros((2, R, PAIRS, 2, B), dtype=np.float32)
        L[0, :, :, 0, :] = A[0]
        L[1, :, :, 1, :] = A[1]
        xbdk = L.reshape(8, PAIRS, 128)
        # pair j = 4t + q -> strip q rows 32q..32q+8 (rows +8..+31 zero),
        # column block t.
        xs = xbdk.reshape(8, NBLK, 4, 128).transpose(2, 0, 1, 3)  # [q,k,t,:]
        ws = wtk.reshape(8, NBLK, 4, DS).transpose(2, 0, 1, 3)
        xw8 = np.concatenate([xs, ws], axis=3)  # [4, 8, NBLK, BLKW]
        xwk = np.zeros((4, 32, NBLK * BLKW), dtype=np.float32)
        xwk[:, :8, :] = xw8.reshape(4, 8, NBLK * BLKW)
        xwk = xwk.reshape(128, NBLK * BLKW).astype(bf16)
        in_maps.append({"xw": np.ascontiguousarray(xwk)})
    return in_maps


TRACE = {"on": False, "last": None}


def kernel(x, pesos):
    from concourse.bass_utils import run_bass_kernel_spmd

    in_maps = _pack_inputs(x, pesos)
    nc = _build_bass()
    res = run_bass_kernel_spmd(
        nc, in_maps, core_ids=list(range(NCORES)), trace=TRACE["on"]
    )
    TRACE["last"] = res
    outs = []
    for k in range(NCORES):
        # Each store chunk s is [128=(c*64+b), npair*512] row-major in DRAM.
        flat = res.results[k]["out"]
        parts = []
        base_elem = 0
        for npair in STORE_PAIRS:
            nelem = npair * 128 * DS
            arr = flat[base_elem:base_elem + nelem].astype(np.float32)
            arr = arr.reshape(2, B, npair, DS).transpose(1, 2, 0, 3)
            parts.append(arr)  # [B, npair, 2, DS]
            base_elem += nelem
        core = np.concatenate(parts, axis=1).reshape(B, NPC, DS)
        outs.append(core)
    full = np.concatenate(outs, axis=1)  # [B, N, DS]
    return np.ascontiguousarray(full).reshape(B, N, D, S)
